# revision 1
# baseline (speedup 1.0000x reference)
"""DeltaSynapse kernel for Trainium2 (8 NeuronCores, SPMD).

Reference computation:
    Xpre[b,e,o] = sum_d delaymap[d,e,o] * Xd[d,b,e]
    I[b,o]      = sum_e (signs*W)[e,o] * Xpre[b,e,o]

Folded:  I[b,o] = sum_{d,e} (delaymap[d,e,o] * Weff[e,o]) * Xd[d,b,e]
i.e. a sum of D matmuls  I += Xd[d] @ (delaymap[d] . Weff).

Sharding: shard the contraction (pre-neuron e) dim across the 8 cores
(256 rows each). Each core reads its own e-slice of delaymap/W/signs/Xd
(~20.1 MiB of fp32 HBM reads, nothing replicated) and produces a full
[16, 2048] partial output; the host sums the 8 partials. Memory-bound:
roofline ~ 20 MiB / ~430 GB/s.

On-chip dtype: fp16. delaymap is one-hot (0/1 -> exact in fp16); W/Xd
lose only 2^-11 rel. SWDGE DMA casts fp32->fp16 in the datapath, so
HBM reads stay fp32 (full bytes) while SBUF tiles halve and the DVE
multiply runs in 2x mode. PE runs fp16 at full rate (1 cyc/row).

Pipeline: delaymap streams in (o-range, e-chunk) slabs, o-major, so
each o-range's 16-matmul PSUM accumulation finishes as soon as its
last slab lands and its output DMA overlaps the remaining stream. The
final o-ranges are half-width so the post-DMA tail is short.
"""

import numpy as np

D, B, N = 8, 16, 2048
NCORES = 8
P = 128                 # SBUF partitions / matmul contraction tile
ESH = N // NCORES       # per-core pre-dim shard = 256
ECH = ESH // P          # e-chunks per core = 2
# output o-ranges: full-width blocks first, narrow at the end so the
# post-DMA elementwise+matmul+output tail is short
O_RANGES = [
    (0, 512),
    (512, 1024),
    (1024, 1536),
    (1536, 1792),
    (1792, 1920),
    (1920, 1984),
    (1984, 2048),
]
# delaymap slabs: one per (o-range, e-chunk), issued o-major
SLABS = [(r, c) for r in range(len(O_RANGES)) for c in range(ECH)]

_prog_cache = {}


def _build_program():
    from concourse import bacc, tile
    from concourse import mybir

    f32 = mybir.dt.float32
    f16 = mybir.dt.float16

    nc = bacc.Bacc(num_swdge_queues=2)
    # Host-prepared layouts (see kernel() below), all fp32 in HBM:
    #   dm{r}_{c}: [P, D, len_r]   delaymap[d, c*128+p, o_range r]
    #   wsa : [P, 2, N]            W/signs rows for e-chunk 0
    #   wsb : [P, 2, N]            W/signs rows for e-chunk 1
    #   xd  : [P, ECH, D, B]       Xd slice transposed
    dms = {}
    for r, c in SLABS:
        o0, o1 = O_RANGES[r]
        dms[(r, c)] = nc.dram_tensor(
            f"dm{r}_{c}", [P, D, o1 - o0], f32, kind="ExternalInput"
        )
    wsa = nc.dram_tensor("wsa", [P, 2, N], f32, kind="ExternalInput")
    wsb = nc.dram_tensor("wsb", [P, 2, N], f32, kind="ExternalInput")
    xd = nc.dram_tensor("xd", [P, ECH, D, B], f32, kind="ExternalInput")
    out = nc.dram_tensor("out", [B, N], f32, kind="ExternalOutput")

    with tile.TileContext(nc) as tc:
        with (
            tc.tile_pool(name="const", bufs=1) as cpool,
            tc.tile_pool(name="dm", bufs=6) as dmpool,
            tc.tile_pool(name="wd", bufs=6) as wdpool,
            tc.tile_pool(name="psum", bufs=1, space="PSUM") as ppool,
            tc.tile_pool(name="outp", bufs=6) as opool,
        ):
            wsa_t = cpool.tile([P, 2, N], f16)
            wsb_t = cpool.tile([P, 2, N], f16)
            weff = cpool.tile([P, ECH, N], f16)
            xd_h = cpool.tile([P, ECH, D, B], f16)

            dm_tiles = {}
            for r, c in SLABS:
                o0, o1 = O_RANGES[r]
                dm_tiles[(r, c)] = dmpool.tile(
                    [P, D, o1 - o0], f16, tag="dmslab", name=f"dm{r}_{c}"
                )

            # SWDGE (gpsimd) DMAs cast fp32->fp16 in the datapath.
            order = [SLABS[0], "wsa", "xd", SLABS[1], "wsb"] + SLABS[2:]
            for item in order:
                if item == "wsa":
                    nc.gpsimd.dma_start(wsa_t[:], wsa[:])
                elif item == "wsb":
                    nc.gpsimd.dma_start(wsb_t[:], wsb[:])
                elif item == "xd":
                    nc.gpsimd.dma_start(xd_h[:], xd[:])
                else:
                    nc.gpsimd.dma_start(dm_tiles[item][:], dms[item][:])

            nc.vector.tensor_mul(weff[:, 0, :], wsa_t[:, 0], wsa_t[:, 1])
            nc.vector.tensor_mul(weff[:, 1, :], wsb_t[:, 0], wsb_t[:, 1])

            psum = ppool.tile([B, N], f32)
            for si, (r, c) in enumerate(SLABS):
                o0, o1 = O_RANGES[r]
                dm_t = dm_tiles[(r, c)]
                wd_t = wdpool.tile([P, D, o1 - o0], f16, tag="wd")
                nc.vector.tensor_mul(
                    wd_t[:],
                    dm_t[:],
                    weff[:, c, o0:o1].unsqueeze(1).broadcast_to(
                        [P, D, o1 - o0]
                    ),
                )
                for d in range(D):
                    nc.tensor.matmul(
                        psum[:, o0:o1],
                        xd_h[:, c, d, :],
                        wd_t[:, d, :],
                        start=(c == 0 and d == 0),
                        stop=(c == ECH - 1 and d == D - 1),
                    )
                # o-range r complete after its last e-chunk: stream it out
                if c == ECH - 1:
                    out_t = opool.tile([B, o1 - o0], f32, tag="out", name=f"o{r}")
                    nc.scalar.copy(out_t[:], psum[:, o0:o1])
                    nc.sync.dma_start(out[:, o0:o1], out_t[:])

    nc.compile()
    return nc


def _get_program():
    if "nc" not in _prog_cache:
        _prog_cache["nc"] = _build_program()
    return _prog_cache["nc"]


def _shard_inputs(Xd, delaymap, W, signs):
    """Pure layout permutation/slicing -> per-core input maps."""
    Xd = np.ascontiguousarray(np.asarray(Xd, dtype=np.float32))
    delaymap = np.asarray(delaymap, dtype=np.float32)
    W = np.asarray(W, dtype=np.float32)
    signs = np.asarray(signs, dtype=np.float32)

    in_maps = []
    for k in range(NCORES):
        esl = slice(k * ESH, (k + 1) * ESH)
        # delaymap [D, ESH, N] -> per-chunk [c][P, D, N], then o-sliced
        dm_cpd = delaymap[:, esl, :].reshape(D, ECH, P, N).transpose(1, 2, 0, 3)
        m = {}
        for r, c in SLABS:
            o0, o1 = O_RANGES[r]
            m[f"dm{r}_{c}"] = np.ascontiguousarray(dm_cpd[c, :, :, o0:o1])
        # W/signs rows for this core's e-slice -> per-chunk [P, 2, N]
        wk = W[esl].reshape(ECH, P, N)
        sk = signs[esl].reshape(ECH, P, N)
        m["wsa"] = np.ascontiguousarray(np.stack([wk[0], sk[0]], axis=1))
        m["wsb"] = np.ascontiguousarray(np.stack([wk[1], sk[1]], axis=1))
        # Xd [D, B, ESH] -> [P, ECH, D, B]
        m["xd"] = np.ascontiguousarray(
            Xd[:, :, esl].reshape(D, B, ECH, P).transpose(3, 2, 0, 1)
        )
        in_maps.append(m)
    return in_maps


def _run(in_maps, trace=False, **kw):
    from concourse.bass_utils import run_bass_kernel_spmd

    nc = _get_program()
    return run_bass_kernel_spmd(nc, in_maps, list(range(NCORES)), trace=trace, **kw)


def _gather(res):
    acc = np.zeros((B, N), dtype=np.float64)
    for k in range(NCORES):
        acc += res.results[k]["out"].astype(np.float64)
    return acc.astype(np.float32)


def kernel(Xd, X, delaymap, W, signs):
    in_maps = _shard_inputs(Xd, delaymap, W, signs)
    return _gather(_run(in_maps))

